# revision 10
# baseline (speedup 1.0000x reference)
"""MoE router gate (group-limited top-k) on 8 Trainium2 NeuronCores.

reference math (per token row of x [T=16384, D=4096], W [E=256, D]):
  logits = x @ W.T                      [T, 256]
  scores = softmax(logits)
  group (8 groups of 32) scores = max of scores per group
  keep top-4 groups, mask rest to -inf
  top-8 experts over masked scores -> indices
  weights = gathered softmax scores, renormalized over the 8 (+1e-9 in denom)

Sharding: data-parallel over tokens, 2048 tokens/core; W.T replicated
(transposed once on host so the contraction dim D lands on SBUF partitions).

Matmul strategy (v2): fp32 matmuls cost 4 cyc/row on the PE; f32r costs
1 cyc/row at free-dim >= 256.  We get fp32-precision logits from f32r
matmuls with an error-compensated 3-term split:
    x = x_hi + x_lo   (x_hi = f32r rounding of x, x_lo exact fp32 residual)
    W = W_hi + W_lo   (same split, done once at startup)
    logits = x_hi*W_hi + x_hi*W_lo + x_lo*W_hi     (drop x_lo*W_lo ~ 2^-2m)
All three products have operands exactly representable in f32r, so the
only error vs fp32 is product/accumulation rounding (~1e-7), same order
as a plain fp32 PE matmul.  The two x_hi terms run as ONE N=512 matmul
against the concatenated [W_hi | W_lo] so LDWEIGHTS stays amortized.

Per-core kernel, for each 128-token tile:
  - DMA x block [128, 4096]
  - PE-transpose x chunks (fp32) -> PSUM, ACT copies PSUM->SBUF twice:
    once fp32 (sfull), once f32r (shi = hardware rounding copy); DVE
    subtracts slo = sfull - shi (exact, fits f32r)
  - f32r matmuls accumulate A[128,512] (= x_hi*[W_hi|W_lo]) and
    B[128,256] (= x_lo*W_hi); DVE folds logits = A_lo + A_hi + B
  - selection on raw logits (softmax is monotone per row), as v1:
    group maxes, group top-4 threshold, masked DVE max/max_index top-8,
    weights = exp(v - M) / (sum8 + 1e-9 * Z) via ACT accum_out
"""

import contextlib

import numpy as np

from concourse import bass, mybir
from concourse.bacc import Bacc
from concourse.tile import TileContext
from concourse.bass_utils import run_bass_kernel_spmd

TOKENS = 16384
DIM = 4096
E = 256
TOPK = 8
G = 8
GSZ = E // G  # 32
NL = 4  # groups kept
N_CORES = 8
TPC = TOKENS // N_CORES  # 2048 tokens per core
NT = TPC // 128  # 16 token tiles per core
KC = DIM // 128  # 32 contraction chunks
NEG_BIG = -1.0e30

_CACHE = {}
_EYE = np.eye(128, dtype=np.float32)


def _build_program(repeat=1):
    nc = Bacc()
    x_ext = nc.declare_dram_parameter("x", [TPC, DIM], mybir.dt.float32, isOutput=False)
    id_ext = nc.declare_dram_parameter(
        "ident", [128, 128], mybir.dt.float32, isOutput=False
    )
    wt_ext = nc.declare_dram_parameter("wt", [DIM, E], mybir.dt.float32, isOutput=False)
    w_out = nc.declare_dram_parameter(
        "weights", [TPC, TOPK], mybir.dt.float32, isOutput=True
    )
    i_out = nc.declare_dram_parameter(
        "indices", [TPC, TOPK], mybir.dt.int32, isOutput=True
    )

    f32 = mybir.dt.float32
    f32r = mybir.dt.float32r

    with TileContext(nc) as tc:
        with (
            tc.tile_pool(name="const", bufs=1) as const_pool,
            tc.tile_pool(name="xb", bufs=3) as xb_pool,
            tc.tile_pool(name="shi", bufs=3) as shi_pool,
            tc.tile_pool(name="slo", bufs=3) as slo_pool,
            tc.tile_pool(name="pxt", bufs=4, space="PSUM") as pxt_pool,
            tc.tile_pool(name="psA", bufs=2, space="PSUM") as psA_pool,
            tc.tile_pool(name="psB", bufs=2, space="PSUM") as psB_pool,
            tc.tile_pool(name="mid", bufs=3) as mid_pool,
            tc.tile_pool(name="small", bufs=3) as small_pool,
        ):
            ident = const_pool.tile([128, 128], f32, tag="ident")
            nc.sync.dma_start(out=ident[:], in_=id_ext[:])

            # W.T resident in SBUF: chunk k occupies columns [k*E, (k+1)*E),
            # partitions = contraction dim d within chunk.  DMA'd in 4
            # pieces so w_cat prep (and the first matmuls) can start
            # before the whole table lands.
            wt_sb = const_pool.tile([128, KC * E], f32, tag="wt")
            for q in range(4):
                kq = KC // 4
                nc.sync.dma_start(
                    out=wt_sb[:, q * kq * E : (q + 1) * kq * E].rearrange(
                        "p (k e) -> p k e", k=kq
                    ),
                    in_=wt_ext[q * kq * 128 : (q + 1) * kq * 128, :].rearrange(
                        "(k p) e -> p k e", p=128
                    ),
                )

            # w_cat: per chunk k, [W_hi[k] (256) | W_lo[k] (256)] as f32r.
            # hi = f32r rounding copy (ACT); lo = wt - hi (DVE, exact fp32
            # bits written through a bitcast view).  Chunked so the first
            # matmuls can start before the whole table is built.
            w_cat = const_pool.tile([128, KC * 512], f32r, tag="wcat")
            for k in range(KC):
                nc.scalar.copy(
                    w_cat[:, k * 512 : k * 512 + 256],
                    wt_sb[:, k * E : (k + 1) * E],
                )
                nc.vector.tensor_sub(
                    w_cat[:, k * 512 + 256 : (k + 1) * 512],
                    wt_sb[:, k * E : (k + 1) * E],
                    w_cat[:, k * 512 : k * 512 + 256].bitcast(f32),
                )

            rep_cm = tc.For_i(0, repeat) if repeat > 1 else contextlib.nullcontext()
            with rep_cm:
              for t in range(NT):
                xb = xb_pool.tile([128, DIM], f32, tag="xb")
                nc.sync.dma_start(out=xb[:], in_=x_ext[t * 128 : (t + 1) * 128, :])

                A = psA_pool.tile([128, 512], f32, tag="A")
                B = psB_pool.tile([128, 256], f32, tag="B")

                # software pipeline: transpose group kb+1 on the PE before
                # the matmuls of group kb, so the ACT rounding copy (shi)
                # and DVE residual (slo) of group kb are ready when the PE
                # reaches its matmuls.
                NG = KC // 4
                shis = [None] * NG
                slos = [None] * NG

                def make_group(kb):
                    pxt = pxt_pool.tile([128, 512], f32, tag="pxt")
                    for j in range(4):
                        k = kb * 4 + j
                        nc.tensor.transpose(
                            pxt[:, j * 128 : (j + 1) * 128],
                            xb[:, k * 128 : (k + 1) * 128],
                            ident[:],
                        )
                    shi = shi_pool.tile([128, 512], f32r, tag="shi")
                    nc.scalar.copy(shi[:], pxt[:])
                    slo = slo_pool.tile([128, 512], f32r, tag="slo")
                    nc.vector.tensor_sub(slo[:], pxt[:], shi[:].bitcast(f32))
                    shis[kb], slos[kb] = shi, slo

                def mm_group(kb):
                    shi, slo = shis[kb], slos[kb]
                    for j in range(4):
                        k = kb * 4 + j
                        nc.tensor.matmul(
                            A[:],
                            lhsT=shi[:, j * 128 : (j + 1) * 128],
                            rhs=w_cat[:, k * 512 : (k + 1) * 512],
                            start=(k == 0),
                            stop=(k == KC - 1),
                        )
                        nc.tensor.matmul(
                            B[:],
                            lhsT=slo[:, j * 128 : (j + 1) * 128],
                            rhs=w_cat[:, k * 512 : k * 512 + 256],
                            start=(k == 0),
                            stop=(k == KC - 1),
                        )

                make_group(0)
                make_group(1)
                for kb in range(NG):
                    if kb + 2 < NG:
                        make_group(kb + 2)
                    mm_group(kb)

                # logits = A_hi_term + A_lo_term + B.  ACT drains PSUM (one
                # reader engine per bank), DVE folds in SBUF.
                acat = mid_pool.tile([128, 512], f32, tag="acat")
                nc.scalar.copy(acat[:], A[:])
                bsb = mid_pool.tile([128, E], f32, tag="bsb")
                nc.scalar.copy(bsb[:], B[:])
                lsum = mid_pool.tile([128, E], f32, tag="lsum")
                nc.vector.tensor_add(lsum[:], acat[:, 0:256], acat[:, 256:512])
                logits = mid_pool.tile([128, E], f32, tag="logits")
                nc.vector.tensor_add(logits[:], lsum[:], bsb[:])

                # ---- selection on raw logits ----
                gs = small_pool.tile([128, G], f32, tag="gs")
                nc.vector.tensor_reduce(
                    gs[:],
                    logits[:].rearrange("p (g e) -> p g e", g=G),
                    axis=mybir.AxisListType.X,
                    op=mybir.AluOpType.max,
                )
                gsort = small_pool.tile([128, 8], f32, tag="gsort")
                nc.vector.max(out=gsort[:], in_=gs[:])
                # bias per group: (gs < 4th-largest) * -1e30
                bias8 = small_pool.tile([128, G], f32, tag="bias8")
                nc.vector.tensor_scalar(
                    bias8[:],
                    gs[:],
                    gsort[:, NL - 1 : NL],
                    NEG_BIG,
                    op0=mybir.AluOpType.is_lt,
                    op1=mybir.AluOpType.mult,
                )
                masked = mid_pool.tile([128, E], f32, tag="masked")
                for g in range(G):
                    nc.vector.tensor_scalar_add(
                        masked[:, g * GSZ : (g + 1) * GSZ],
                        logits[:, g * GSZ : (g + 1) * GSZ],
                        bias8[:, g : g + 1],
                    )
                vals8 = small_pool.tile([128, 8], f32, tag="vals8")
                nc.vector.max(out=vals8[:], in_=masked[:])
                idx8 = small_pool.tile([128, 8], mybir.dt.uint32, tag="idx8")
                nc.vector.max_index(out=idx8[:], in_max=vals8[:], in_values=masked[:])

                # ---- weights: e_k / (S + 1e-9 * Z), shifted by M = top value
                negm = small_pool.tile([128, 1], f32, tag="negm")
                nc.vector.tensor_scalar_mul(negm[:], vals8[:, 0:1], -1.0)
                scr = mid_pool.tile([128, E], f32, tag="scr")
                zfull = small_pool.tile([128, 1], f32, tag="zfull")
                nc.scalar.activation(
                    scr[:],
                    logits[:],
                    mybir.ActivationFunctionType.Exp,
                    bias=negm[:],
                    accum_out=zfull[:],
                )
                e8 = small_pool.tile([128, 8], f32, tag="e8")
                s8 = small_pool.tile([128, 1], f32, tag="s8")
                nc.scalar.activation(
                    e8[:],
                    vals8[:],
                    mybir.ActivationFunctionType.Exp,
                    bias=negm[:],
                    accum_out=s8[:],
                )
                den = small_pool.tile([128, 1], f32, tag="den")
                nc.vector.tensor_scalar(
                    den[:],
                    zfull[:],
                    1.0e-9,
                    None,
                    op0=mybir.AluOpType.mult,
                )
                nc.vector.tensor_add(den[:], den[:], s8[:])
                rcp = small_pool.tile([128, 1], f32, tag="rcp")
                nc.vector.reciprocal(rcp[:], den[:])
                w8 = small_pool.tile([128, 8], f32, tag="w8")
                nc.vector.tensor_scalar_mul(w8[:], e8[:], rcp[:])
                i32 = small_pool.tile([128, 8], mybir.dt.int32, tag="i32")
                nc.vector.tensor_copy(out=i32[:], in_=idx8[:])

                nc.sync.dma_start(
                    out=w_out[t * 128 : (t + 1) * 128, :], in_=w8[:]
                )
                nc.sync.dma_start(
                    out=i_out[t * 128 : (t + 1) * 128, :], in_=i32[:]
                )
    return nc


def get_program(repeat=1):
    key = ("nc", repeat)
    if key not in _CACHE:
        nc = _build_program(repeat)
        # Bacc defers register allocation + wait-splitting to finalize();
        # the PJRT path serializes the module as-is, so lower it now.
        nc.finalize()
        _CACHE[key] = nc
    return _CACHE[key]


def kernel(x: np.ndarray, weight: np.ndarray, repeat=1, **run_kwargs):
    x = np.ascontiguousarray(x, dtype=np.float32)
    wt = np.ascontiguousarray(weight.T, dtype=np.float32)  # [DIM, E]
    nc = get_program(repeat)
    in_maps = [
        {"x": x[c * TPC : (c + 1) * TPC], "wt": wt, "ident": _EYE}
        for c in range(N_CORES)
    ]
    res = run_bass_kernel_spmd(nc, in_maps, list(range(N_CORES)), **run_kwargs)
    weights = np.concatenate([res.results[c]["weights"] for c in range(N_CORES)], axis=0)
    indices = np.concatenate([res.results[c]["indices"] for c in range(N_CORES)], axis=0)
    _CACHE["last_results"] = res
    return weights.astype(np.float32), indices.astype(np.int32)


# revision 11
# speedup vs baseline: 1.2417x; 1.2417x over previous
"""MoE router gate (group-limited top-k) on 8 Trainium2 NeuronCores.

reference math (per token row of x [T=16384, D=4096], W [E=256, D]):
  logits = x @ W.T                      [T, 256]
  scores = softmax(logits)
  group (8 groups of 32) scores = max of scores per group
  keep top-4 groups, mask rest to -inf
  top-8 experts over masked scores -> indices
  weights = gathered softmax scores, renormalized over the 8 (+1e-9 in denom)

Sharding: data-parallel over tokens, 2048 tokens/core; W.T replicated
(transposed once on host so the contraction dim D lands on SBUF partitions).

Matmul strategy (v2): fp32 matmuls cost 4 cyc/row on the PE; f32r costs
1 cyc/row at free-dim >= 256.  We get fp32-precision logits from f32r
matmuls with an error-compensated 3-term split:
    x = x_hi + x_lo   (x_hi = f32r rounding of x, x_lo exact fp32 residual)
    W = W_hi + W_lo   (same split, done once at startup)
    logits = x_hi*W_hi + x_hi*W_lo + x_lo*W_hi     (drop x_lo*W_lo ~ 2^-2m)
All three products have operands exactly representable in f32r, so the
only error vs fp32 is product/accumulation rounding (~1e-7), same order
as a plain fp32 PE matmul.  The two x_hi terms run as ONE N=512 matmul
against the concatenated [W_hi | W_lo] so LDWEIGHTS stays amortized.

Per-core kernel, for each 128-token tile:
  - DMA x block [128, 4096]
  - PE-transpose x chunks (fp32) -> PSUM pxt; ACT makes shi = f32r
    rounding copy (PSUM->SBUF); DVE subtracts slo = pxt - shi (exact
    residual, fits f32r losslessly)
  - f32r matmuls accumulate A[128,512] (= x_hi*[W_hi|W_lo]) and
    B[128,256] (= x_lo*W_hi); ACT drains A,B to SBUF, DVE folds
    logits = A_lo + A_hi + B
  - transpose group kb+2 is emitted before matmul group kb (software
    pipeline) so shi/slo are always ready when the PE reaches them
  - selection on raw logits (softmax is monotone per row), as v1:
    group maxes, group top-4 threshold, masked DVE max/max_index top-8,
    weights = exp(v - M) / (sum8 + 1e-9 * Z) via ACT accum_out

PE cost/tile: 32 fp32 transposes (256 cyc) + 32 N=512 + 32 N=256 f32r
matmuls (1 cyc/row) = 32768 cyc vs 40960 for the fp32 baseline; cost
model (TimelineSim) 254.9us vs 316.3us baseline (which measured 263.2us
on HW via REPEAT differencing => calibrated estimate ~212us).
"""

import contextlib

import numpy as np

from concourse import bass, mybir
from concourse.bacc import Bacc
from concourse.tile import TileContext
from concourse.bass_utils import run_bass_kernel_spmd

TOKENS = 16384
DIM = 4096
E = 256
TOPK = 8
G = 8
GSZ = E // G  # 32
NL = 4  # groups kept
N_CORES = 8
TPC = TOKENS // N_CORES  # 2048 tokens per core
NT = TPC // 128  # 16 token tiles per core
KC = DIM // 128  # 32 contraction chunks
NEG_BIG = -1.0e30

_CACHE = {}
_EYE = np.eye(128, dtype=np.float32)


def _build_program(repeat=1):
    nc = Bacc()
    x_ext = nc.declare_dram_parameter("x", [TPC, DIM], mybir.dt.float32, isOutput=False)
    id_ext = nc.declare_dram_parameter(
        "ident", [128, 128], mybir.dt.float32, isOutput=False
    )
    wt_ext = nc.declare_dram_parameter("wt", [DIM, E], mybir.dt.float32, isOutput=False)
    w_out = nc.declare_dram_parameter(
        "weights", [TPC, TOPK], mybir.dt.float32, isOutput=True
    )
    i_out = nc.declare_dram_parameter(
        "indices", [TPC, TOPK], mybir.dt.int32, isOutput=True
    )

    f32 = mybir.dt.float32
    f32r = mybir.dt.float32r

    with TileContext(nc) as tc:
        with (
            tc.tile_pool(name="const", bufs=1) as const_pool,
            tc.tile_pool(name="xb", bufs=3) as xb_pool,
            tc.tile_pool(name="shi", bufs=3) as shi_pool,
            tc.tile_pool(name="slo", bufs=3) as slo_pool,
            tc.tile_pool(name="pxt", bufs=4, space="PSUM") as pxt_pool,
            tc.tile_pool(name="psA", bufs=2, space="PSUM") as psA_pool,
            tc.tile_pool(name="psB", bufs=2, space="PSUM") as psB_pool,
            tc.tile_pool(name="mid", bufs=3) as mid_pool,
            tc.tile_pool(name="small", bufs=3) as small_pool,
        ):
            ident = const_pool.tile([128, 128], f32, tag="ident")
            nc.sync.dma_start(out=ident[:], in_=id_ext[:])

            # W.T resident in SBUF: chunk k occupies columns [k*E, (k+1)*E),
            # partitions = contraction dim d within chunk.  DMA'd in 4
            # pieces so w_cat prep (and the first matmuls) can start
            # before the whole table lands.
            wt_sb = const_pool.tile([128, KC * E], f32, tag="wt")
            for q in range(4):
                kq = KC // 4
                nc.sync.dma_start(
                    out=wt_sb[:, q * kq * E : (q + 1) * kq * E].rearrange(
                        "p (k e) -> p k e", k=kq
                    ),
                    in_=wt_ext[q * kq * 128 : (q + 1) * kq * 128, :].rearrange(
                        "(k p) e -> p k e", p=128
                    ),
                )

            # w_cat: per chunk k, [W_hi[k] (256) | W_lo[k] (256)] as f32r.
            # hi = f32r rounding copy (ACT); lo = wt - hi (DVE, exact fp32
            # bits written through a bitcast view).  Chunked so the first
            # matmuls can start before the whole table is built.
            w_cat = const_pool.tile([128, KC * 512], f32r, tag="wcat")
            for k in range(KC):
                nc.scalar.copy(
                    w_cat[:, k * 512 : k * 512 + 256],
                    wt_sb[:, k * E : (k + 1) * E],
                )
                nc.vector.tensor_sub(
                    w_cat[:, k * 512 + 256 : (k + 1) * 512],
                    wt_sb[:, k * E : (k + 1) * E],
                    w_cat[:, k * 512 : k * 512 + 256].bitcast(f32),
                )

            rep_cm = tc.For_i(0, repeat) if repeat > 1 else contextlib.nullcontext()
            with rep_cm:
              for t in range(NT):
                xb = xb_pool.tile([128, DIM], f32, tag="xb")
                nc.sync.dma_start(out=xb[:], in_=x_ext[t * 128 : (t + 1) * 128, :])

                A = psA_pool.tile([128, 512], f32, tag="A")
                B = psB_pool.tile([128, 256], f32, tag="B")

                # software pipeline: transpose group kb+1 on the PE before
                # the matmuls of group kb, so the ACT rounding copy (shi)
                # and DVE residual (slo) of group kb are ready when the PE
                # reaches its matmuls.
                NG = KC // 4
                shis = [None] * NG
                slos = [None] * NG

                def make_group(kb):
                    pxt = pxt_pool.tile([128, 512], f32, tag="pxt")
                    for j in range(4):
                        k = kb * 4 + j
                        nc.tensor.transpose(
                            pxt[:, j * 128 : (j + 1) * 128],
                            xb[:, k * 128 : (k + 1) * 128],
                            ident[:],
                        )
                    shi = shi_pool.tile([128, 512], f32r, tag="shi")
                    nc.scalar.copy(shi[:], pxt[:])
                    slo = slo_pool.tile([128, 512], f32r, tag="slo")
                    nc.vector.tensor_sub(slo[:], pxt[:], shi[:].bitcast(f32))
                    shis[kb], slos[kb] = shi, slo

                def mm_group(kb):
                    shi, slo = shis[kb], slos[kb]
                    for j in range(4):
                        k = kb * 4 + j
                        nc.tensor.matmul(
                            A[:],
                            lhsT=shi[:, j * 128 : (j + 1) * 128],
                            rhs=w_cat[:, k * 512 : (k + 1) * 512],
                            start=(k == 0),
                            stop=(k == KC - 1),
                        )
                        nc.tensor.matmul(
                            B[:],
                            lhsT=slo[:, j * 128 : (j + 1) * 128],
                            rhs=w_cat[:, k * 512 : k * 512 + 256],
                            start=(k == 0),
                            stop=(k == KC - 1),
                        )

                make_group(0)
                make_group(1)
                for kb in range(NG):
                    if kb + 2 < NG:
                        make_group(kb + 2)
                    mm_group(kb)

                # logits = A_hi_term + A_lo_term + B.  ACT drains PSUM (one
                # reader engine per bank), DVE folds in SBUF.
                acat = mid_pool.tile([128, 512], f32, tag="acat")
                nc.scalar.copy(acat[:], A[:])
                bsb = mid_pool.tile([128, E], f32, tag="bsb")
                nc.scalar.copy(bsb[:], B[:])
                lsum = mid_pool.tile([128, E], f32, tag="lsum")
                nc.vector.tensor_add(lsum[:], acat[:, 0:256], acat[:, 256:512])
                logits = mid_pool.tile([128, E], f32, tag="logits")
                nc.vector.tensor_add(logits[:], lsum[:], bsb[:])

                # ---- selection on raw logits ----
                gs = small_pool.tile([128, G], f32, tag="gs")
                nc.vector.tensor_reduce(
                    gs[:],
                    logits[:].rearrange("p (g e) -> p g e", g=G),
                    axis=mybir.AxisListType.X,
                    op=mybir.AluOpType.max,
                )
                gsort = small_pool.tile([128, 8], f32, tag="gsort")
                nc.vector.max(out=gsort[:], in_=gs[:])
                # bias per group: (gs < 4th-largest) * -1e30
                bias8 = small_pool.tile([128, G], f32, tag="bias8")
                nc.vector.tensor_scalar(
                    bias8[:],
                    gs[:],
                    gsort[:, NL - 1 : NL],
                    NEG_BIG,
                    op0=mybir.AluOpType.is_lt,
                    op1=mybir.AluOpType.mult,
                )
                masked = mid_pool.tile([128, E], f32, tag="masked")
                for g in range(G):
                    nc.vector.tensor_scalar_add(
                        masked[:, g * GSZ : (g + 1) * GSZ],
                        logits[:, g * GSZ : (g + 1) * GSZ],
                        bias8[:, g : g + 1],
                    )
                vals8 = small_pool.tile([128, 8], f32, tag="vals8")
                nc.vector.max(out=vals8[:], in_=masked[:])
                idx8 = small_pool.tile([128, 8], mybir.dt.uint32, tag="idx8")
                nc.vector.max_index(out=idx8[:], in_max=vals8[:], in_values=masked[:])

                # ---- weights: e_k / (S + 1e-9 * Z), shifted by M = top value
                negm = small_pool.tile([128, 1], f32, tag="negm")
                nc.vector.tensor_scalar_mul(negm[:], vals8[:, 0:1], -1.0)
                scr = mid_pool.tile([128, E], f32, tag="scr")
                zfull = small_pool.tile([128, 1], f32, tag="zfull")
                nc.scalar.activation(
                    scr[:],
                    logits[:],
                    mybir.ActivationFunctionType.Exp,
                    bias=negm[:],
                    accum_out=zfull[:],
                )
                e8 = small_pool.tile([128, 8], f32, tag="e8")
                s8 = small_pool.tile([128, 1], f32, tag="s8")
                nc.scalar.activation(
                    e8[:],
                    vals8[:],
                    mybir.ActivationFunctionType.Exp,
                    bias=negm[:],
                    accum_out=s8[:],
                )
                den = small_pool.tile([128, 1], f32, tag="den")
                nc.vector.tensor_scalar(
                    den[:],
                    zfull[:],
                    1.0e-9,
                    None,
                    op0=mybir.AluOpType.mult,
                )
                nc.vector.tensor_add(den[:], den[:], s8[:])
                rcp = small_pool.tile([128, 1], f32, tag="rcp")
                nc.vector.reciprocal(rcp[:], den[:])
                w8 = small_pool.tile([128, 8], f32, tag="w8")
                nc.vector.tensor_scalar_mul(w8[:], e8[:], rcp[:])
                i32 = small_pool.tile([128, 8], mybir.dt.int32, tag="i32")
                nc.vector.tensor_copy(out=i32[:], in_=idx8[:])

                nc.sync.dma_start(
                    out=w_out[t * 128 : (t + 1) * 128, :], in_=w8[:]
                )
                nc.sync.dma_start(
                    out=i_out[t * 128 : (t + 1) * 128, :], in_=i32[:]
                )
    return nc


def get_program(repeat=1):
    key = ("nc", repeat)
    if key not in _CACHE:
        nc = _build_program(repeat)
        # Bacc defers register allocation + wait-splitting to finalize();
        # the PJRT path serializes the module as-is, so lower it now.
        nc.finalize()
        _CACHE[key] = nc
    return _CACHE[key]


def kernel(x: np.ndarray, weight: np.ndarray, repeat=1, **run_kwargs):
    x = np.ascontiguousarray(x, dtype=np.float32)
    wt = np.ascontiguousarray(weight.T, dtype=np.float32)  # [DIM, E]
    nc = get_program(repeat)
    in_maps = [
        {"x": x[c * TPC : (c + 1) * TPC], "wt": wt, "ident": _EYE}
        for c in range(N_CORES)
    ]
    res = run_bass_kernel_spmd(nc, in_maps, list(range(N_CORES)), **run_kwargs)
    weights = np.concatenate([res.results[c]["weights"] for c in range(N_CORES)], axis=0)
    indices = np.concatenate([res.results[c]["indices"] for c in range(N_CORES)], axis=0)
    _CACHE["last_results"] = res
    return weights.astype(np.float32), indices.astype(np.int32)
